# revision 1
# baseline (speedup 1.0000x reference)
"""Multi-head attention (B=4, T=2048, C=1024, H=16, D=64) on 8 TRN2 cores.

Sharding: core i handles batch b=i//2 and the 8 heads of half hh=i%2.
Each core computes its heads' contribution through the row-sharded output
projection -> partial y [T, C]; host sums the two partials per batch.

Per-core layouts (host pre-arranged):
  xT  [C, T]    = x[b].T
  wq/wk/wv [C, 512]  columns = (local head)*64 + d
  wpt [512, C]  rows  = (local head)*64 + d   (= Wp.T row-slice)
  bp  [C]       bias on even cores, zeros on odd (summed partials)

On-chip dataflow per core:
  qT/kT [2h*64=128, T] via lhsT=w-chunk, rhs=xT-chunk (f32r, N=512)
  v     [s,d] natural via lhsT=xT s-slice, rhs=wv-chunk (N=256)
  scoresT[s,t]: lhsT=kT s-block [64,128], rhs=qT t-tile [64,512],
                2 heads row-tiled (K=64 each, partitions 0-63 / 64-127)
  exp on ScalarE PSUM->SBUF with scale=1/sqrt(C); causal: restrict to the
  valid t-range, one constant [128,128] mask multiply on straddling blocks
  PV: lhsT=[v ; ones] [128,65], rhs=pT -> outT [65,512] PSUM accumulated
  over s-blocks; row 64 = softmax normalizer Z
  normalize: DVE reciprocal(Z) -> gpsimd partition_broadcast -> DVE mult
  y: lhsT=outcatT [c,t-block], rhs=wpt [c, c'] + bias, DMA out
"""

import os
import sys

import numpy as np

for _p in ("/opt/trn_rl_repo", "/root/.axon_site/_ro/trn_rl_repo"):
    if os.path.isdir(_p) and _p not in sys.path:
        sys.path.append(_p)

import concourse.bass as bass
import concourse.bacc as bacc
import concourse.mybir as mybir
import concourse.tile as tile
from concourse.bass_utils import run_bass_kernel_spmd

B, T, C, H, D = 4, 2048, 1024, 16, 64
HL = H // 2          # heads per core
P = 128
NCH = C // P         # 8 c-chunks
NTT = T // 512       # 4 t-tiles of 512
NSB = T // P         # 16 s-blocks of 128
SCALE = 1.0 / 32.0   # 1/sqrt(C)

F32 = mybir.dt.float32
F32R = mybir.dt.float32r


def _build(causal: bool, debug: bool = False) -> bass.Bass:
    nc = bacc.Bacc("TRN2", target_bir_lowering=False, debug=False, num_devices=8)

    xT = nc.dram_tensor("xT", [C, T], F32R, kind="ExternalInput").ap()
    wq_d = nc.dram_tensor("wq", [C, HL * D], F32R, kind="ExternalInput").ap()
    wk_d = nc.dram_tensor("wk", [C, HL * D], F32R, kind="ExternalInput").ap()
    wv_d = nc.dram_tensor("wv", [C, HL * D], F32R, kind="ExternalInput").ap()
    wpt_d = nc.dram_tensor("wpt", [HL * D, C], F32R, kind="ExternalInput").ap()
    bp_d = nc.dram_tensor("bp", [C], F32, kind="ExternalInput").ap()
    y_d = nc.dram_tensor("y", [T, C], F32, kind="ExternalOutput").ap()
    dbg = {}
    if debug:
        dbg["q"] = nc.dram_tensor("dbg_q", [2, P, T], F32, kind="ExternalOutput").ap()
        dbg["k"] = nc.dram_tensor("dbg_k", [2, P, T], F32, kind="ExternalOutput").ap()
        dbg["v"] = nc.dram_tensor("dbg_v", [P, NSB * 4 * (D + 1)], F32, kind="ExternalOutput").ap()
        dbg["oc"] = nc.dram_tensor("dbg_oc", [4, P, T], F32, kind="ExternalOutput").ap()

    with tile.TileContext(nc) as tc:
        _emit(nc, tc, causal, xT, wq_d, wk_d, wv_d, wpt_d, bp_d, y_d, dbg)
    nc.compile()
    return nc


def _emit(nc, tc, causal, xT, wq_d, wk_d, wv_d, wpt_d, bp_d, y_d, dbg={}):
    from contextlib import ExitStack

    ctx = ExitStack()
    with ctx:
        consts = ctx.enter_context(tc.tile_pool(name="consts", bufs=1))
        q_pool = ctx.enter_context(tc.tile_pool(name="qT", bufs=3))
        k_pool = ctx.enter_context(tc.tile_pool(name="kT", bufs=3))
        v_pool = ctx.enter_context(tc.tile_pool(name="v", bufs=2))
        oc_pool = ctx.enter_context(tc.tile_pool(name="outcat", bufs=4))
        p_pool = ctx.enter_context(tc.tile_pool(name="pT", bufs=3))
        z_pool = ctx.enter_context(tc.tile_pool(name="zb", bufs=2))
        rzb_pool = ctx.enter_context(tc.tile_pool(name="rzb", bufs=2))
        psA = ctx.enter_context(tc.tile_pool(name="psA", bufs=2, space="PSUM"))
        psB = ctx.enter_context(tc.tile_pool(name="psB", bufs=2, space="PSUM"))
        pso = ctx.enter_context(tc.tile_pool(name="pso", bufs=2, space="PSUM"))

        # constant [128, 2, 128] additive causal mask: 0 where free>=partition
        # else -1e9 (two copies along the middle dim, one per row-tiled head)
        mask = None
        if causal:
            mask = consts.tile([P, 2, P], F32)
            nc.vector.memset(mask, 0.0)
            for _u in range(2):
                nc.gpsimd.affine_select(
                    out=mask[:, _u, :], in_=mask[:, _u, :],
                    compare_op=mybir.AluOpType.is_ge,
                    fill=-1e9, base=0,
                    pattern=[[1, P]], channel_multiplier=-1,
                )

        ones_bc = consts.tile([P, P], F32R)
        nc.vector.memset(ones_bc.bitcast(F32), 1.0)

        outcat = [oc_pool.tile([P, T], F32R, tag="outcat", name=f"outcat{i}")
                  for i in range(4)]

        inner = ExitStack()
        with inner:
            wq_pool = inner.enter_context(tc.tile_pool(name="wq", bufs=1))
            wk_pool = inner.enter_context(tc.tile_pool(name="wk", bufs=1))
            wv_pool = inner.enter_context(tc.tile_pool(name="wv", bufs=1))
            x_pool = inner.enter_context(tc.tile_pool(name="xh", bufs=1))

            for hg in range(2):  # head-group of 4 heads (= 2 pairs)
                hsl = slice(hg * 4 * D, (hg + 1) * 4 * D)
                wq_t = wq_pool.tile([P, NCH, 4 * D], F32R, tag="wq")
                wk_t = wk_pool.tile([P, NCH, 4 * D], F32R, tag="wk")
                wv_t = wv_pool.tile([P, NCH, 4 * D], F32R, tag="wv")
                for w_t, w_d in ((wq_t, wq_d), (wk_t, wk_d), (wv_t, wv_d)):
                    nc.sync.dma_start(
                        out=w_t,
                        in_=w_d[:, hsl].rearrange("(n p) d -> p n d", p=P),
                    )

                qT2 = [q_pool.tile([P, T], F32R, tag="qT", name=f"qT{i}")
                       for i in range(2)]
                kT2 = [k_pool.tile([P, T], F32R, tag="kT", name=f"kT{i}")
                       for i in range(2)]
                # v: [s-part, s-block, head-in-group, d + ones]
                v_t = v_pool.tile([P, NSB, 4, D + 1], F32R, tag="v")
                nc.vector.memset(v_t[:, :, :, D:].bitcast(F32), 1.0)

                for th in range(2):  # t/s halves of 1024
                    xh = x_pool.tile([P, NCH, 1024], F32R, tag="xh")
                    for c in range(NCH):
                        nc.sync.dma_start(
                            out=xh[:, c, :],
                            in_=xT[c * P:(c + 1) * P, th * 1024:(th + 1) * 1024],
                        )
                    tg = slice(th * 1024, (th + 1) * 1024)
                    # ---- q/k projections ----
                    for pr in range(2):
                        wsl = slice(pr * P, (pr + 1) * P)
                        qps = psA.tile([P, 2, 512], F32, tag="psA", name="qps")
                        kps = psA.tile([P, 2, 512], F32, tag="psA", name="kps")
                        for c in range(NCH):
                            for tt in range(2):
                                nc.tensor.matmul(
                                    qps[:, tt, :], wq_t[:, c, wsl],
                                    xh[:, c, tt * 512:(tt + 1) * 512],
                                    start=c == 0, stop=c == NCH - 1)
                                nc.tensor.matmul(
                                    kps[:, tt, :], wk_t[:, c, wsl],
                                    xh[:, c, tt * 512:(tt + 1) * 512],
                                    start=c == 0, stop=c == NCH - 1)
                        nc.vector.tensor_copy(
                            out=qT2[pr][:, tg],
                            in_=qps.rearrange("p u t -> p (u t)"))
                        nc.vector.tensor_copy(
                            out=kT2[pr][:, tg],
                            in_=kps.rearrange("p u t -> p (u t)"))
                    # ---- v projection (natural [s, d]) ----
                    for sbp in range(4):
                        vps = psB.tile([P, 2, 256], F32, tag="psB", name="vps")
                        for c in range(NCH):
                            for u in range(2):
                                nc.tensor.matmul(
                                    vps[:, u, :],
                                    xh[:, c, (sbp * 2 + u) * P:(sbp * 2 + u + 1) * P],
                                    wv_t[:, c, :],
                                    start=(c == 0 and u == 0), stop=c == NCH - 1)
                        sb0 = th * 8 + sbp * 2
                        nc.vector.tensor_copy(
                            out=v_t[:, sb0:sb0 + 2, :, 0:D],
                            in_=vps.rearrange("p u (h d) -> p u h d", h=4),
                        )

                if dbg and hg == 0:
                    for pr2 in range(2):
                        nc.sync.dma_start(out=dbg["q"][pr2], in_=qT2[pr2].bitcast(F32))
                        nc.sync.dma_start(out=dbg["k"][pr2], in_=kT2[pr2].bitcast(F32))
                    nc.sync.dma_start(
                        out=dbg["v"],
                        in_=v_t.rearrange("p a b c -> p (a b c)").bitcast(F32))

                # ---- attention for this head-group ----
                for pr in range(2):
                    pair = hg * 2 + pr
                    zb = z_pool.tile([P, 3, 512], F32, tag="zb", name="zb")
                    nc.vector.memset(zb, 1.0)
                    for j in range(NTT):
                        nsb_j = 4 * (j + 1) if causal else NSB
                        outp = [pso.tile([D + 1, 512], F32, tag="pso",
                                         name=f"outp{i}") for i in range(2)]
                        def emit_pv(i, lo, last):
                            for u in range(2):
                                nc.tensor.matmul(
                                    outp[u][:, lo:512],
                                    v_t[:, i, pr * 2 + u, :],
                                    pend[i][:, u, lo:512],
                                    start=(i == 0), stop=last,
                                    skip_group_check=True)
                            del pend[i]

                        pend = {}
                        prev = None
                        for i in range(nsb_j):
                            r = i - 4 * j if causal else -1
                            lo = max(r, 0) * P
                            last = i == nsb_j - 1
                            scs = psA.tile([P, 2, 512], F32, tag="psA", name="scs")
                            pts = p_pool.tile([P, 2, 512], F32R, tag="pT", name="pts")
                            pend[i] = pts
                            for u in range(2):
                                dsl = slice(u * D, (u + 1) * D)
                                nc.tensor.matmul(
                                    scs[:, u, :],
                                    kT2[pr][dsl, i * P:(i + 1) * P],
                                    qT2[pr][dsl, j * 512:(j + 1) * 512],
                                    start=True, stop=True)
                            if causal and r >= 0:
                                nc.vector.tensor_add(
                                    scs[:, :, lo:lo + P],
                                    scs[:, :, lo:lo + P],
                                    mask)
                            nc.scalar.activation(
                                out=pts[:, :, lo:512],
                                in_=scs[:, :, lo:512],
                                func=mybir.ActivationFunctionType.Exp,
                                scale=SCALE)
                            if prev is not None:
                                emit_pv(*prev)
                            prev = (i, lo, last)
                        if prev is not None:
                            emit_pv(*prev)
                        for u in range(2):
                            # raw (unnormalized) head output + Z row gather
                            nc.vector.tensor_copy(
                                out=outcat[pair][u * D:(u + 1) * D,
                                                 j * 512:(j + 1) * 512],
                                in_=outp[u][0:D, :])
                            idx = j * 2 + u
                            nc.vector.tensor_copy(
                                out=zb[32 * (idx // 3):32 * (idx // 3) + 1,
                                       idx % 3, :],
                                in_=outp[u][D:D + 1, :])
                    # batched normalizer: one reciprocal for all 8 (j, u)
                    # rows, then per-row broadcast via K=1 matmul into PSUM
                    rzb_all = z_pool.tile([P, 3, 512], F32R, tag="zb", name="rz_all")
                    with nc.allow_low_precision(reason="softmax normalizer"):
                        nc.vector.reciprocal(out=rzb_all, in_=zb)
                    for j in range(NTT):
                        for u in range(2):
                            idx = j * 2 + u
                            k0 = 32 * (idx // 3)
                            bps = pso.tile([P, 512], F32, tag="pso", name="bps")
                            nc.tensor.matmul(
                                bps,
                                ones_bc[k0:k0 + 1, :],
                                rzb_all[k0:k0 + 1, idx % 3, :],
                                start=True, stop=True)
                            osl = outcat[pair][u * D:(u + 1) * D,
                                               j * 512:(j + 1) * 512]
                            nc.vector.tensor_mul(
                                osl, osl.bitcast(F32),
                                bps[u * D:(u + 1) * D, :])

        if dbg:
            for q2 in range(4):
                nc.sync.dma_start(out=dbg["oc"][q2], in_=outcat[q2].bitcast(F32))

        # ---- output projection ----
        wpt_pool = ctx.enter_context(tc.tile_pool(name="wpt", bufs=4))
        bpb_pool = ctx.enter_context(tc.tile_pool(name="bpb", bufs=1))
        yst_pool = ctx.enter_context(tc.tile_pool(name="yst", bufs=3))
        wpt_t = [wpt_pool.tile([P, C], F32R, tag="wpt", name=f"wpt{i}") for i in range(4)]
        for q in range(4):
            nc.sync.dma_start(out=wpt_t[q], in_=wpt_d[q * P:(q + 1) * P, :])
        bpb = bpb_pool.tile([P, C], F32)
        nc.sync.dma_start(
            out=bpb,
            in_=bass.AP(tensor=bp_d.tensor, offset=0, ap=[[0, P], [1, C]]),
        )
        for m in range(T // P):
            for n in range(2):
                yps = psB.tile([P, 512], F32, tag="psB", name="yps")
                for q in range(4):
                    nc.tensor.matmul(
                        yps,
                        outcat[q][:, m * P:(m + 1) * P],
                        wpt_t[q][:, n * 512:(n + 1) * 512],
                        start=(q == 0), stop=(q == 3))
                yt = yst_pool.tile([P, 512], F32, tag="yst", name="yt")
                nc.vector.tensor_add(yt, yps, bpb[:, n * 512:(n + 1) * 512])
                nc.sync.dma_start(
                    out=y_d[m * P:(m + 1) * P, n * 512:(n + 1) * 512],
                    in_=yt)


_NC_CACHE = {}
LAST_RESULTS = None


def kernel(x, Wq, Wk, Wv, Wp, bp, is_masked, **_unused):
    global LAST_RESULTS
    x = np.asarray(x, np.float32)
    Wq = np.asarray(Wq, np.float32)
    Wk = np.asarray(Wk, np.float32)
    Wv = np.asarray(Wv, np.float32)
    Wp = np.asarray(Wp, np.float32)
    bp = np.asarray(bp, np.float32)
    causal = bool(np.asarray(is_masked).item())

    if causal not in _NC_CACHE:
        _NC_CACHE[causal] = _build(causal)
    nc = _NC_CACHE[causal]

    # host-side layout prep
    wq_r = np.ascontiguousarray(Wq.transpose(1, 0, 2).reshape(C, H * D))
    wk_r = np.ascontiguousarray(Wk.transpose(1, 0, 2).reshape(C, H * D))
    wv_r = np.ascontiguousarray(Wv.transpose(1, 0, 2).reshape(C, H * D))
    wpt = np.ascontiguousarray(Wp.T)
    zeros = np.zeros_like(bp)

    xTs = [np.ascontiguousarray(x[b].T) for b in range(B)]
    in_maps = []
    for core in range(8):
        b, hh = core // 2, core % 2
        csl = slice(hh * HL * D, (hh + 1) * HL * D)
        in_maps.append({
            "xT": xTs[b],
            "wq": np.ascontiguousarray(wq_r[:, csl]),
            "wk": np.ascontiguousarray(wk_r[:, csl]),
            "wv": np.ascontiguousarray(wv_r[:, csl]),
            "wpt": np.ascontiguousarray(wpt[csl, :]),
            "bp": bp if hh == 0 else zeros,
        })

    trace = bool(int(os.environ.get("KERNEL_TRACE", "0")))
    res = run_bass_kernel_spmd(
        nc, in_maps, core_ids=list(range(8)), trace=trace)
    LAST_RESULTS = res

    y = np.empty((B, T, C), np.float32)
    for b in range(B):
        y[b] = res.results[2 * b]["y"] + res.results[2 * b + 1]["y"]
    return y



# revision 2
# speedup vs baseline: 1.2381x; 1.2381x over previous
"""Multi-head attention (B=4, T=2048, C=1024, H=16, D=64) on 8 TRN2 cores. v2.

Sharding: core i handles batch b=i//2 and heads of half hh=i%2 (8 heads =
4 pairs). Row-sharded output projection -> partial y [T, C]; host sums the
two partials per batch.

v2 dataflow (per core):
  q/k projections: fp8e4 DoubleRow (weights prescaled x32 on host), psum f32
    -> DVE cast to fp8 staging [128,(u d), t] -> DMA rearrange into the
    d-split quadrant layout qkdr[32*pr+p, qk, kt, u, t] (d = kt*32 + p).
  scores: fp8 DoubleRow over the d-split (K=32 x 2 ktiles), quadrant
    tile_position, t-range trimmed to the causal block range.
  exp on ScalarE psum->bf16 with scale 1/32768; diagonal blocks zeroed
    post-exp on GpSimd (affine_select, t<s -> 0).
  PV: bf16, lhsT=[v|ones] [128,65], transposed accumulate; row 64 = Z.
  normalize: Z rows DMA'd psum->sbuf, reciprocal_approx_fast, rz broadcast
    via stride-0 DMA, fused DVE mult psum->outcat bf16.
  y: bf16 matmuls over 4 pair-chunks + DVE bias add, DMA out f32.
  Weave: projection/v/y pieces are emitted as fillers inside the ACT-bound
  attention stream to keep the PE busy.
"""

import os
import sys
from collections import deque

import numpy as np
import ml_dtypes

for _p in ("/opt/trn_rl_repo", "/root/.axon_site/_ro/trn_rl_repo"):
    if os.path.isdir(_p) and _p not in sys.path:
        sys.path.append(_p)

import concourse.bass as bass
import concourse.bacc as bacc
import concourse.mybir as mybir
import concourse.tile as tile
from concourse.bass_utils import run_bass_kernel_spmd

B, T, C, H, D = 4, 2048, 1024, 16, 64
HL = H // 2          # heads per core
P = 128
NTT = T // 512       # 4 t-tiles of 512
NSB = T // P         # 16 s-blocks of 128
SCALE_DR = 1.0 / 32768.0   # 1/sqrt(C) / (32*32)

F32 = mybir.dt.float32
F32R = mybir.dt.float32r
BF16 = mybir.dt.bfloat16
F8 = mybir.dt.float8e4
DR = mybir.MatmulPerfMode.DoubleRow
EXP = mybir.ActivationFunctionType.Exp

E4 = ml_dtypes.float8_e4m3
BF = ml_dtypes.bfloat16


def _build(causal: bool) -> bass.Bass:
    nc = bacc.Bacc("TRN2", target_bir_lowering=False, debug=False, num_devices=8)

    x8_d = nc.dram_tensor("x8", [C, T], F8, kind="ExternalInput").ap()
    x16_d = nc.dram_tensor("x16", [C, T], BF16, kind="ExternalInput").ap()
    wq8_d = nc.dram_tensor("wq8", [C, HL * D], F8, kind="ExternalInput").ap()
    wk8_d = nc.dram_tensor("wk8", [C, HL * D], F8, kind="ExternalInput").ap()
    wv16_d = nc.dram_tensor("wv16", [C, HL * D], BF16, kind="ExternalInput").ap()
    wpt16_d = nc.dram_tensor("wpt16", [HL * D, C], BF16, kind="ExternalInput").ap()
    bp_d = nc.dram_tensor("bp", [C], F32, kind="ExternalInput").ap()
    y_d = nc.dram_tensor("y", [T, C], F32, kind="ExternalOutput").ap()

    with tile.TileContext(nc) as tc:
        _emit(nc, tc, causal, x8_d, x16_d, wq8_d, wk8_d, wv16_d, wpt16_d,
              bp_d, y_d)
    nc.compile()
    return nc


def _emit(nc, tc, causal, x8_d, x16_d, wq8_d, wk8_d, wv16_d, wpt16_d,
          bp_d, y_d):
    from contextlib import ExitStack

    ctx = ExitStack()
    with ctx:
        x8_pool = ctx.enter_context(tc.tile_pool(name="x8", bufs=1))
        x16_pool = ctx.enter_context(tc.tile_pool(name="x16", bufs=1))
        w_pool = ctx.enter_context(tc.tile_pool(name="w", bufs=1))
        qkdr_pool = ctx.enter_context(tc.tile_pool(name="qkdr", bufs=1))
        stage_pool = ctx.enter_context(tc.tile_pool(name="stage", bufs=2))
        v_pool = ctx.enter_context(tc.tile_pool(name="v16", bufs=4))
        oc_pool = ctx.enter_context(tc.tile_pool(name="outcat", bufs=4))
        p_pool = ctx.enter_context(tc.tile_pool(name="pts", bufs=3))
        z_pool = ctx.enter_context(tc.tile_pool(name="zb", bufs=2))
        rz_pool = ctx.enter_context(tc.tile_pool(name="rz", bufs=2))
        bps_pool = ctx.enter_context(tc.tile_pool(name="bps", bufs=2))
        yst_pool = ctx.enter_context(tc.tile_pool(name="yst", bufs=3))
        psS = ctx.enter_context(tc.tile_pool(name="psS", bufs=2, space="PSUM"))
        psO = ctx.enter_context(tc.tile_pool(name="psO", bufs=3, space="PSUM"))
        psA = ctx.enter_context(tc.tile_pool(name="psA", bufs=1, space="PSUM"))

        # ---- weight / bias / x DMAs ----
        wq8_t = w_pool.tile([P, 8, HL * D], F8, tag="wq8")
        wk8_t = w_pool.tile([P, 8, HL * D], F8, tag="wk8")
        wv16_t = w_pool.tile([P, 8, HL * D], BF16, tag="wv16")
        for w_t, w_d in ((wq8_t, wq8_d), (wk8_t, wk8_d), (wv16_t, wv16_d)):
            nc.sync.dma_start(
                out=w_t, in_=w_d.rearrange("(n p) d -> p n d", p=P))
        wpt16_t = w_pool.tile([P, 4, C], BF16, tag="wpt16")
        nc.sync.dma_start(
            out=wpt16_t, in_=wpt16_d.rearrange("(n p) d -> p n d", p=P))
        bpb = w_pool.tile([P, C], F32, tag="bpb")
        nc.sync.dma_start(
            out=bpb,
            in_=bass.AP(tensor=bp_d.tensor, offset=0, ap=[[0, P], [1, C]]))

        x8_t = x8_pool.tile([P, 8, T], F8, tag="x8")
        x16_t = x16_pool.tile([P, 8, T], BF16, tag="x16")
        for ch in range(8):
            for th in range(2):
                tg = slice(th * 1024, (th + 1) * 1024)
                nc.sync.dma_start(
                    out=x8_t[:, ch, tg],
                    in_=x8_d[ch * P:(ch + 1) * P, tg])
        for ch in range(8):
            for th in range(2):
                tg = slice(th * 1024, (th + 1) * 1024)
                nc.sync.dma_start(
                    out=x16_t[:, ch, tg],
                    in_=x16_d[ch * P:(ch + 1) * P, tg])

        # qkdr[32*pr+p, qk, kt, u, t] fp8, d = kt*32 + p per (pair, head u)
        qkdr = qkdr_pool.tile([P, 2, 2, 2, T], F8, tag="qkdr")

        # v: 4 groups of 4 s-blocks: [s-part, sb%4, head, d | ones]
        v16_g = [v_pool.tile([P, 4, HL, D + 1], BF16, tag="v16",
                             name=f"v16_{g}") for g in range(4)]
        for g in range(4):
            nc.vector.memset(v16_g[g][:, :, :, D:], 1.0)

        outcat = [oc_pool.tile([P, T], BF16, tag="outcat", name=f"oc{q}")
                  for q in range(4)]

        ones_bc16 = w_pool.tile([1, P], BF16, tag="ones")
        nc.vector.memset(ones_bc16, 1.0)

        # ---------- emit helpers ----------
        def emit_proj_piece(pr, th, qk):
            w_t = wq8_t if qk == 0 else wk8_t
            stage = stage_pool.tile([P, 1024], F8, tag="stage")
            for tt in range(2):
                ps = psA.tile([P, 512], F32, tag="psA", name="qkps")
                for sub in range(2):
                    n0 = th * 1024 + tt * 512 + sub * 256
                    for cp in range(4):
                        nc.tensor.matmul(
                            ps[:, sub * 256:(sub + 1) * 256],
                            w_t[:, 2 * cp:2 * cp + 2, pr * P:(pr + 1) * P],
                            x8_t[:, 2 * cp:2 * cp + 2, n0:n0 + 256],
                            start=(cp == 0), stop=(cp == 3), perf_mode=DR)
                nc.vector.tensor_copy(
                    out=stage[:, tt * 512:(tt + 1) * 512], in_=ps)
            for kt in range(2):
                for u in range(2):
                    base = u * D + kt * 32
                    nc.sync.dma_start(
                        out=qkdr[32 * pr:32 * pr + 32, qk, kt, u,
                                 th * 1024:(th + 1) * 1024],
                        in_=stage[base:base + 32, :])

        def emit_v_piece(sb):
            ps = psA.tile([P, 512], F32, tag="psA", name="vps")
            for ch in range(8):
                nc.tensor.matmul(
                    ps, x16_t[:, ch, sb * P:(sb + 1) * P], wv16_t[:, ch, :],
                    start=(ch == 0), stop=(ch == 7))
            nc.vector.tensor_copy(
                out=v16_g[sb // 4][:, sb % 4, :, 0:D],
                in_=ps.rearrange("p (h d) -> p h d", h=HL))

        def emit_y_piece(m, n):
            yps = psA.tile([P, 512], F32, tag="psA", name="yps")
            for q in range(4):
                nc.tensor.matmul(
                    yps, outcat[q][:, m * P:(m + 1) * P],
                    wpt16_t[:, q, n * 512:(n + 1) * 512],
                    start=(q == 0), stop=(q == 3))
            yt = yst_pool.tile([P, 512], F32, tag="yst")
            nc.vector.tensor_add(yt, yps, bpb[:, n * 512:(n + 1) * 512])
            nc.sync.dma_start(
                out=y_d[m * P:(m + 1) * P, n * 512:(n + 1) * 512], in_=yt)

        fillers = deque()

        def pump(k=1):
            for _ in range(k):
                if fillers:
                    fillers.popleft()()

        def drain():
            while fillers:
                fillers.popleft()()

        def emit_norm(pr, j, outp):
            # Z rows (bf16) -> broadcast Z via bf16 ones-matmul -> reciprocal
            # of the broadcast on DVE -> normalize multiply on GpSimd.
            zbb = z_pool.tile([1, 2, 512], BF16, tag="zb")
            for u in range(2):
                nc.vector.tensor_copy(out=zbb[:, u, :], in_=outp[u][D:D + 1, :])
            bzp = psA.tile([P, 512], F32, tag="psA", name="bzp")
            for u in range(2):
                nc.tensor.matmul(
                    bzp[u * D:(u + 1) * D, :], ones_bc16[:, 0:D],
                    zbb[:, u, :], start=True, stop=True,
                    tile_position=(0, u * D))
            bz = bps_pool.tile([P, 512], F32, tag="bps")
            nc.vector.reciprocal_approx_fast(out=bz, in_=bzp)
            for u in range(2):
                osl = outcat[pr][u * D:(u + 1) * D, j * 512:(j + 1) * 512]
                nc.vector.tensor_copy(out=osl, in_=outp[u][0:D, :])
                nc.gpsimd.tensor_mul(osl, osl, bz[u * D:(u + 1) * D, :])

        def emit_attention_pair(pr):
            for j in range(NTT):
                nsb_j = 4 * (j + 1) if causal else NSB
                outp = [psO.tile([D + 1, 512], F32, tag="psO",
                                 name=f"outp{pr}_{j}_{u}") for u in range(2)]

                def emit_pv(i, lo, last):
                    pts = pend.pop(i)
                    for u in range(2):
                        nc.tensor.matmul(
                            outp[u][:, lo:512],
                            v16_g[i // 4][:, i % 4, pr * 2 + u, :],
                            pts[:, u, lo:512],
                            start=(i == 0), stop=last,
                            skip_group_check=True)

                pend = {}
                prev = None
                for i in range(nsb_j):
                    r = i - 4 * j if causal else -1
                    lo = max(r, 0) * P
                    scs = psS.tile([P, 2, 512], F32, tag="scs")
                    pts = p_pool.tile([P, 2, 512], BF16, tag="pts")
                    pend[i] = pts
                    for u in range(2):
                        t0 = lo
                        while t0 < 512:
                            t1 = min(t0 + 256, 512)
                            nc.tensor.matmul(
                                scs[:, u, t0:t1],
                                qkdr[32 * pr:32 * pr + 32, 1, :, u,
                                     i * P:(i + 1) * P],
                                qkdr[32 * pr:32 * pr + 32, 0, :, u,
                                     j * 512 + t0:j * 512 + t1],
                                start=True, stop=True, perf_mode=DR,
                                tile_position=(32 * pr, 0))
                            t0 = t1
                    nc.scalar.activation(
                        out=pts[:, :, lo:512], in_=scs[:, :, lo:512],
                        func=EXP, scale=SCALE_DR)
                    if causal and r >= 0:
                        nc.gpsimd.affine_select(
                            out=pts[:, :, lo:lo + P],
                            in_=pts[:, :, lo:lo + P],
                            compare_op=mybir.AluOpType.is_ge,
                            fill=0.0, base=0,
                            pattern=[[0, 2], [1, P]], channel_multiplier=-1)
                    if prev is not None:
                        emit_pv(*prev)
                        pump(1)
                    prev = (i, lo, i == nsb_j - 1)
                emit_pv(*prev)
                pump(1)
                emit_norm(pr, j, outp)
                if pr == 3:
                    jj = j
                    for m in range(4 * jj, 4 * jj + 4):
                        for n in range(2):
                            fillers.append(
                                lambda m=m, n=n: emit_y_piece(m, n))

        # ---------- schedule ----------
        # proj pair 0 + first v group up front
        for th in range(2):
            for qk in range(2):
                emit_proj_piece(0, th, qk)
        for sb in range(4):
            emit_v_piece(sb)

        for pr in range(4):
            # queue fillers: remaining v groups (pair 0 only) then next proj
            if pr == 0:
                for sb in range(4, NSB):
                    fillers.append(lambda sb=sb: emit_v_piece(sb))
            if pr < 3:
                for th in range(2):
                    for qk in range(2):
                        fillers.append(
                            lambda pr2=pr + 1, th=th, qk=qk:
                            emit_proj_piece(pr2, th, qk))
            if pr > 0:
                # everything pair pr reads must be emitted before its reads
                drain()
            emit_attention_pair(pr)
        drain()


_NC_CACHE = {}
LAST_RESULTS = None


def kernel(x, Wq, Wk, Wv, Wp, bp, is_masked, **_unused):
    global LAST_RESULTS
    x = np.asarray(x, np.float32)
    Wq = np.asarray(Wq, np.float32)
    Wk = np.asarray(Wk, np.float32)
    Wv = np.asarray(Wv, np.float32)
    Wp = np.asarray(Wp, np.float32)
    bp = np.asarray(bp, np.float32)
    causal = bool(np.asarray(is_masked).item())

    if causal not in _NC_CACHE:
        _NC_CACHE[causal] = _build(causal)
    nc = _NC_CACHE[causal]

    wq_r = Wq.transpose(1, 0, 2).reshape(C, H * D)
    wk_r = Wk.transpose(1, 0, 2).reshape(C, H * D)
    wv_r = Wv.transpose(1, 0, 2).reshape(C, H * D)
    wpt = np.ascontiguousarray(Wp.T)
    zeros = np.zeros_like(bp)

    in_maps = []
    for core in range(8):
        b, hh = core // 2, core % 2
        csl = slice(hh * HL * D, (hh + 1) * HL * D)
        xT = np.ascontiguousarray(x[b].T)
        in_maps.append({
            "x8": xT.astype(E4),
            "x16": xT.astype(BF),
            "wq8": np.ascontiguousarray(32.0 * wq_r[:, csl]).astype(E4),
            "wk8": np.ascontiguousarray(32.0 * wk_r[:, csl]).astype(E4),
            "wv16": np.ascontiguousarray(wv_r[:, csl]).astype(BF),
            "wpt16": np.ascontiguousarray(wpt[csl, :]).astype(BF),
            "bp": bp if hh == 0 else zeros,
        })

    trace = bool(int(os.environ.get("KERNEL_TRACE", "0")))
    res = run_bass_kernel_spmd(
        nc, in_maps, core_ids=list(range(8)), trace=trace)
    LAST_RESULTS = res

    y = np.empty((B, T, C), np.float32)
    for b in range(B):
        y[b] = res.results[2 * b]["y"] + res.results[2 * b + 1]["y"]
    return y


# revision 3
# speedup vs baseline: 1.3052x; 1.0542x over previous
"""Multi-head attention (B=4, T=2048, C=1024, H=16, D=64) on 8 TRN2 cores. v2.

Sharding: core i handles batch b=i//2 and heads of half hh=i%2 (8 heads =
4 pairs). Row-sharded output projection -> partial y [T, C]; host sums the
two partials per batch.

v2 dataflow (per core):
  q/k projections: fp8e4 DoubleRow (weights prescaled x32 on host), psum f32
    -> DVE cast to fp8 staging [128,(u d), t] -> DMA rearrange into the
    d-split quadrant layout qkdr[32*pr+p, qk, kt, u, t] (d = kt*32 + p).
  scores: fp8 DoubleRow over the d-split (K=32 x 2 ktiles), quadrant
    tile_position, t-range trimmed to the causal block range.
  exp on ScalarE psum->bf16 with scale 1/32768; diagonal blocks zeroed
    post-exp on GpSimd (affine_select, t<s -> 0).
  PV: bf16, lhsT=[v|ones] [128,65], transposed accumulate; row 64 = Z.
  normalize: Z rows DMA'd psum->sbuf, reciprocal_approx_fast, rz broadcast
    via stride-0 DMA, fused DVE mult psum->outcat bf16.
  y: bf16 matmuls over 4 pair-chunks + DVE bias add, DMA out f32.
  Weave: projection/v/y pieces are emitted as fillers inside the ACT-bound
  attention stream to keep the PE busy.
"""

import os
import sys
from collections import deque

import numpy as np
import ml_dtypes

for _p in ("/opt/trn_rl_repo", "/root/.axon_site/_ro/trn_rl_repo"):
    if os.path.isdir(_p) and _p not in sys.path:
        sys.path.append(_p)

import concourse.bass as bass
import concourse.bacc as bacc
import concourse.mybir as mybir
import concourse.tile as tile
from concourse.bass_utils import run_bass_kernel_spmd

B, T, C, H, D = 4, 2048, 1024, 16, 64
HL = H // 2          # heads per core
P = 128
NTT = T // 512       # 4 t-tiles of 512
NSB = T // P         # 16 s-blocks of 128
SCALE_DR = 1.0 / 32768.0   # 1/sqrt(C) / (32*32)

F32 = mybir.dt.float32
F32R = mybir.dt.float32r
BF16 = mybir.dt.bfloat16
F8 = mybir.dt.float8e4
DR = mybir.MatmulPerfMode.DoubleRow
EXP = mybir.ActivationFunctionType.Exp

E4 = ml_dtypes.float8_e4m3
BF = ml_dtypes.bfloat16


def _build(causal: bool) -> bass.Bass:
    nc = bacc.Bacc("TRN2", target_bir_lowering=False, debug=False, num_devices=8)

    x8_d = nc.dram_tensor("x8", [C, T], F8, kind="ExternalInput").ap()
    x16_d = nc.dram_tensor("x16", [C, T], BF16, kind="ExternalInput").ap()
    wq8_d = nc.dram_tensor("wq8", [C, HL * D], F8, kind="ExternalInput").ap()
    wk8_d = nc.dram_tensor("wk8", [C, HL * D], F8, kind="ExternalInput").ap()
    wv16_d = nc.dram_tensor("wv16", [C, HL * D], BF16, kind="ExternalInput").ap()
    wpt16_d = nc.dram_tensor("wpt16", [HL * D, C], BF16, kind="ExternalInput").ap()
    bp_d = nc.dram_tensor("bp", [C], F32, kind="ExternalInput").ap()
    y_d = nc.dram_tensor("y", [T, C], F32, kind="ExternalOutput").ap()

    with tile.TileContext(nc) as tc:
        _emit(nc, tc, causal, x8_d, x16_d, wq8_d, wk8_d, wv16_d, wpt16_d,
              bp_d, y_d)
    nc.compile()
    return nc


def _emit(nc, tc, causal, x8_d, x16_d, wq8_d, wk8_d, wv16_d, wpt16_d,
          bp_d, y_d):
    from contextlib import ExitStack

    ctx = ExitStack()
    with ctx:
        x8_pool = ctx.enter_context(tc.tile_pool(name="x8", bufs=1))
        x16_pool = ctx.enter_context(tc.tile_pool(name="x16", bufs=1))
        w_pool = ctx.enter_context(tc.tile_pool(name="w", bufs=1))
        qkdr_pool = ctx.enter_context(tc.tile_pool(name="qkdr", bufs=8))
        stage_pool = ctx.enter_context(tc.tile_pool(name="stage", bufs=2))
        v_pool = ctx.enter_context(tc.tile_pool(name="v16", bufs=4))
        oc_pool = ctx.enter_context(tc.tile_pool(name="outcat", bufs=4))
        p_pool = ctx.enter_context(tc.tile_pool(name="pts", bufs=3))
        z_pool = ctx.enter_context(tc.tile_pool(name="zb", bufs=2))
        rz_pool = ctx.enter_context(tc.tile_pool(name="rz", bufs=2))
        bps_pool = ctx.enter_context(tc.tile_pool(name="bps", bufs=2))
        yst_pool = ctx.enter_context(tc.tile_pool(name="yst", bufs=3))
        psS = ctx.enter_context(tc.tile_pool(name="psS", bufs=2, space="PSUM"))
        psO = ctx.enter_context(tc.tile_pool(name="psO", bufs=3, space="PSUM"))
        psA = ctx.enter_context(tc.tile_pool(name="psA", bufs=1, space="PSUM"))

        # ---- weight / bias / x DMAs ----
        wq8_t = w_pool.tile([P, 8, HL * D], F8, tag="wq8")
        wk8_t = w_pool.tile([P, 8, HL * D], F8, tag="wk8")
        wv16_t = w_pool.tile([P, 8, HL * D], BF16, tag="wv16")
        for w_t, w_d in ((wq8_t, wq8_d), (wk8_t, wk8_d), (wv16_t, wv16_d)):
            nc.sync.dma_start(
                out=w_t, in_=w_d.rearrange("(n p) d -> p n d", p=P))
        wpt16_t = w_pool.tile([P, 4, C], BF16, tag="wpt16")
        nc.sync.dma_start(
            out=wpt16_t, in_=wpt16_d.rearrange("(n p) d -> p n d", p=P))
        bpb = w_pool.tile([P, C], F32, tag="bpb")
        nc.sync.dma_start(
            out=bpb,
            in_=bass.AP(tensor=bp_d.tensor, offset=0, ap=[[0, P], [1, C]]))

        x8_t = x8_pool.tile([P, 8, T], F8, tag="x8")
        x16_t = x16_pool.tile([P, 8, T], BF16, tag="x16")
        for ch in range(8):
            for th in range(2):
                tg = slice(th * 1024, (th + 1) * 1024)
                nc.sync.dma_start(
                    out=x8_t[:, ch, tg],
                    in_=x8_d[ch * P:(ch + 1) * P, tg])
        for ch in range(8):
            for th in range(2):
                tg = slice(th * 1024, (th + 1) * 1024)
                nc.sync.dma_start(
                    out=x16_t[:, ch, tg],
                    in_=x16_d[ch * P:(ch + 1) * P, tg])

        # persistent fp8 q/k per pair: [(u d), t]
        qk8 = [[qkdr_pool.tile([P, T], F8, tag="qk8", name=f"qk8_{pr}_{qk}")
                for qk in range(2)] for pr in range(4)]

        # v: 4 groups of 4 s-blocks: [s-part, sb%4, head, d | ones]
        v16_g = [v_pool.tile([P, 4, HL, D + 1], BF16, tag="v16",
                             name=f"v16_{g}") for g in range(4)]
        for g in range(4):
            nc.vector.memset(v16_g[g][:, :, :, D:], 1.0)

        outcat = [oc_pool.tile([P, T], BF16, tag="outcat", name=f"oc{q}")
                  for q in range(4)]

        ones_bc16 = w_pool.tile([1, P], BF16, tag="ones")
        nc.vector.memset(ones_bc16, 1.0)

        # ---------- emit helpers ----------
        def emit_proj_piece(pr, th, qk):
            w_t = wq8_t if qk == 0 else wk8_t
            stage = qk8[pr][qk]
            for tt in range(2):
                ps = psA.tile([P, 512], F32, tag="psA", name="qkps")
                for sub in range(2):
                    n0 = th * 1024 + tt * 512 + sub * 256
                    for cp in range(4):
                        nc.tensor.matmul(
                            ps[:, sub * 256:(sub + 1) * 256],
                            w_t[:, 2 * cp:2 * cp + 2, pr * P:(pr + 1) * P],
                            x8_t[:, 2 * cp:2 * cp + 2, n0:n0 + 256],
                            start=(cp == 0), stop=(cp == 3), perf_mode=DR)
                nc.vector.tensor_copy(
                    out=stage[:, th * 1024 + tt * 512:
                              th * 1024 + (tt + 1) * 512], in_=ps)

        def emit_v_piece(sb):
            ps = psA.tile([P, 512], F32, tag="psA", name="vps")
            for ch in range(8):
                nc.tensor.matmul(
                    ps, x16_t[:, ch, sb * P:(sb + 1) * P], wv16_t[:, ch, :],
                    start=(ch == 0), stop=(ch == 7))
            nc.vector.tensor_copy(
                out=v16_g[sb // 4][:, sb % 4, :, 0:D],
                in_=ps.rearrange("p (h d) -> p h d", h=HL))

        def emit_y_piece(m, n):
            yps = psA.tile([P, 512], F32, tag="psA", name="yps")
            for q in range(4):
                nc.tensor.matmul(
                    yps, outcat[q][:, m * P:(m + 1) * P],
                    wpt16_t[:, q, n * 512:(n + 1) * 512],
                    start=(q == 0), stop=(q == 3))
            yt = yst_pool.tile([P, 512], F32, tag="yst")
            nc.vector.tensor_add(yt, yps, bpb[:, n * 512:(n + 1) * 512])
            nc.sync.dma_start(
                out=y_d[m * P:(m + 1) * P, n * 512:(n + 1) * 512], in_=yt)

        fillers = deque()

        def pump(k=1):
            for _ in range(k):
                if fillers:
                    fillers.popleft()()

        def drain():
            while fillers:
                fillers.popleft()()

        def emit_norm(pr, j, outp):
            # Z rows (bf16) -> broadcast Z via bf16 ones-matmul -> reciprocal
            # of the broadcast on DVE -> normalize multiply on GpSimd.
            zbb = z_pool.tile([1, 2, 512], BF16, tag="zb")
            for u in range(2):
                nc.vector.tensor_copy(out=zbb[:, u, :], in_=outp[u][D:D + 1, :])
            bzp = psA.tile([P, 512], F32, tag="psA", name="bzp")
            for u in range(2):
                nc.tensor.matmul(
                    bzp[u * D:(u + 1) * D, :], ones_bc16[:, 0:D],
                    zbb[:, u, :], start=True, stop=True,
                    tile_position=(0, u * D))
            bz = bps_pool.tile([P, 512], F32, tag="bps")
            nc.vector.reciprocal_approx_fast(out=bz, in_=bzp)
            for u in range(2):
                osl = outcat[pr][u * D:(u + 1) * D, j * 512:(j + 1) * 512]
                nc.vector.tensor_copy(out=osl, in_=outp[u][0:D, :])
                nc.gpsimd.tensor_mul(osl, osl, bz[u * D:(u + 1) * D, :])

        def emit_attention_pair(pr):
            for j in range(NTT):
                nsb_j = 4 * (j + 1) if causal else NSB
                outp = [psO.tile([D + 1, 512], F32, tag="psO",
                                 name=f"outp{pr}_{j}_{u}") for u in range(2)]

                def emit_pv(i, lo, last):
                    pts = pend.pop(i)
                    for u in range(2):
                        nc.tensor.matmul(
                            outp[u][:, lo:512],
                            v16_g[i // 4][:, i % 4, pr * 2 + u, :],
                            pts[:, u, lo:512],
                            start=(i == 0), stop=last,
                            skip_group_check=True)

                pend = {}
                prev = None
                for i in range(nsb_j):
                    r = i - 4 * j if causal else -1
                    lo = max(r, 0) * P
                    scs = psS.tile([P, 2, 512], F32, tag="scs")
                    pts = p_pool.tile([P, 2, 512], BF16, tag="pts")
                    pend[i] = pts
                    for u in range(2):
                        dsl = slice(u * D, (u + 1) * D)
                        nc.tensor.matmul(
                            scs[:, u, lo:512],
                            qk8[pr][1][dsl, i * P:(i + 1) * P],
                            qk8[pr][0][dsl, j * 512 + lo:(j + 1) * 512],
                            start=True, stop=True)
                    nc.scalar.activation(
                        out=pts[:, :, lo:512], in_=scs[:, :, lo:512],
                        func=EXP, scale=SCALE_DR)
                    if causal and r >= 0:
                        nc.gpsimd.affine_select(
                            out=pts[:, :, lo:lo + P],
                            in_=pts[:, :, lo:lo + P],
                            compare_op=mybir.AluOpType.is_ge,
                            fill=0.0, base=0,
                            pattern=[[0, 2], [1, P]], channel_multiplier=-1)
                    if prev is not None:
                        emit_pv(*prev)
                        pump(1)
                    prev = (i, lo, i == nsb_j - 1)
                emit_pv(*prev)
                pump(1)
                emit_norm(pr, j, outp)
                if pr == 3:
                    jj = j
                    for m in range(4 * jj, 4 * jj + 4):
                        for n in range(2):
                            fillers.append(
                                lambda m=m, n=n: emit_y_piece(m, n))

        # ---------- schedule ----------
        # proj pair 0 + first v group up front
        for th in range(2):
            for qk in range(2):
                emit_proj_piece(0, th, qk)
        for sb in range(4):
            emit_v_piece(sb)

        for pr in range(4):
            # queue fillers: remaining v groups (pair 0 only) then next proj
            if pr == 0:
                for sb in range(4, NSB):
                    fillers.append(lambda sb=sb: emit_v_piece(sb))
            if pr < 3:
                for th in range(2):
                    for qk in range(2):
                        fillers.append(
                            lambda pr2=pr + 1, th=th, qk=qk:
                            emit_proj_piece(pr2, th, qk))
            if pr > 0:
                # everything pair pr reads must be emitted before its reads
                drain()
            emit_attention_pair(pr)
        drain()


_NC_CACHE = {}
LAST_RESULTS = None


def kernel(x, Wq, Wk, Wv, Wp, bp, is_masked, **_unused):
    global LAST_RESULTS
    x = np.asarray(x, np.float32)
    Wq = np.asarray(Wq, np.float32)
    Wk = np.asarray(Wk, np.float32)
    Wv = np.asarray(Wv, np.float32)
    Wp = np.asarray(Wp, np.float32)
    bp = np.asarray(bp, np.float32)
    causal = bool(np.asarray(is_masked).item())

    if causal not in _NC_CACHE:
        _NC_CACHE[causal] = _build(causal)
    nc = _NC_CACHE[causal]

    wq_r = Wq.transpose(1, 0, 2).reshape(C, H * D)
    wk_r = Wk.transpose(1, 0, 2).reshape(C, H * D)
    wv_r = Wv.transpose(1, 0, 2).reshape(C, H * D)
    wpt = np.ascontiguousarray(Wp.T)
    zeros = np.zeros_like(bp)

    in_maps = []
    for core in range(8):
        b, hh = core // 2, core % 2
        csl = slice(hh * HL * D, (hh + 1) * HL * D)
        xT = np.ascontiguousarray(x[b].T)
        in_maps.append({
            "x8": xT.astype(E4),
            "x16": xT.astype(BF),
            "wq8": np.ascontiguousarray(32.0 * wq_r[:, csl]).astype(E4),
            "wk8": np.ascontiguousarray(32.0 * wk_r[:, csl]).astype(E4),
            "wv16": np.ascontiguousarray(wv_r[:, csl]).astype(BF),
            "wpt16": np.ascontiguousarray(wpt[csl, :]).astype(BF),
            "bp": bp if hh == 0 else zeros,
        })

    trace = bool(int(os.environ.get("KERNEL_TRACE", "0")))
    res = run_bass_kernel_spmd(
        nc, in_maps, core_ids=list(range(8)), trace=trace)
    LAST_RESULTS = res

    y = np.empty((B, T, C), np.float32)
    for b in range(B):
        y[b] = res.results[2 * b]["y"] + res.results[2 * b + 1]["y"]
    return y


# revision 4
# speedup vs baseline: 1.3489x; 1.0335x over previous
"""Multi-head attention (B=4, T=2048, C=1024, H=16, D=64) on 8 TRN2 cores. v2.

Sharding: core i handles batch b=i//2 and heads of half hh=i%2 (8 heads =
4 pairs). Row-sharded output projection -> partial y [T, C]; host sums the
two partials per batch.

v2 dataflow (per core):
  q/k projections: fp8e4 DoubleRow (weights prescaled x32 on host), psum f32
    -> DVE cast to fp8 staging [128,(u d), t] -> DMA rearrange into the
    d-split quadrant layout qkdr[32*pr+p, qk, kt, u, t] (d = kt*32 + p).
  scores: fp8 DoubleRow over the d-split (K=32 x 2 ktiles), quadrant
    tile_position, t-range trimmed to the causal block range.
  exp on ScalarE psum->bf16 with scale 1/32768; diagonal blocks zeroed
    post-exp on GpSimd (affine_select, t<s -> 0).
  PV: bf16, lhsT=[v|ones] [128,65], transposed accumulate; row 64 = Z.
  normalize: Z rows DMA'd psum->sbuf, reciprocal_approx_fast, rz broadcast
    via stride-0 DMA, fused DVE mult psum->outcat bf16.
  y: bf16 matmuls over 4 pair-chunks + DVE bias add, DMA out f32.
  Weave: projection/v/y pieces are emitted as fillers inside the ACT-bound
  attention stream to keep the PE busy.
"""

import os
import sys
from collections import deque

import numpy as np
import ml_dtypes

for _p in ("/opt/trn_rl_repo", "/root/.axon_site/_ro/trn_rl_repo"):
    if os.path.isdir(_p) and _p not in sys.path:
        sys.path.append(_p)

import concourse.bass as bass
import concourse.bacc as bacc
import concourse.mybir as mybir
import concourse.tile as tile
from concourse.bass_utils import run_bass_kernel_spmd

B, T, C, H, D = 4, 2048, 1024, 16, 64
HL = H // 2          # heads per core
P = 128
NTT = T // 512       # 4 t-tiles of 512
NSB = T // P         # 16 s-blocks of 128
SCALE_DR = 1.0 / 32768.0   # 1/sqrt(C) / (32*32)

F32 = mybir.dt.float32
F32R = mybir.dt.float32r
BF16 = mybir.dt.bfloat16
F8 = mybir.dt.float8e4
DR = mybir.MatmulPerfMode.DoubleRow
EXP = mybir.ActivationFunctionType.Exp

E4 = ml_dtypes.float8_e4m3
BF = ml_dtypes.bfloat16


def _build(causal: bool) -> bass.Bass:
    nc = bacc.Bacc("TRN2", target_bir_lowering=False, debug=False, num_devices=8)

    x8_d = nc.dram_tensor("x8", [C, T], F8, kind="ExternalInput").ap()
    x16_d = nc.dram_tensor("x16", [C, T], BF16, kind="ExternalInput").ap()
    wq8_d = nc.dram_tensor("wq8", [C, HL * D], F8, kind="ExternalInput").ap()
    wk8_d = nc.dram_tensor("wk8", [C, HL * D], F8, kind="ExternalInput").ap()
    wv16_d = nc.dram_tensor("wv16", [C, HL * D], BF16, kind="ExternalInput").ap()
    wpt16_d = nc.dram_tensor("wpt16", [HL * D, C], BF16, kind="ExternalInput").ap()
    bp_d = nc.dram_tensor("bp", [C], F32, kind="ExternalInput").ap()
    y_d = nc.dram_tensor("y", [T, C], F32, kind="ExternalOutput").ap()

    with tile.TileContext(nc) as tc:
        _emit(nc, tc, causal, x8_d, x16_d, wq8_d, wk8_d, wv16_d, wpt16_d,
              bp_d, y_d)
    nc.compile()
    return nc


def _emit(nc, tc, causal, x8_d, x16_d, wq8_d, wk8_d, wv16_d, wpt16_d,
          bp_d, y_d):
    from contextlib import ExitStack

    ctx = ExitStack()
    with ctx:
        x8_pool = ctx.enter_context(tc.tile_pool(name="x8", bufs=1))
        x16_pool = ctx.enter_context(tc.tile_pool(name="x16", bufs=1))
        w_pool = ctx.enter_context(tc.tile_pool(name="w", bufs=1))
        qkdr_pool = ctx.enter_context(tc.tile_pool(name="qkdr", bufs=8))
        stage_pool = ctx.enter_context(tc.tile_pool(name="stage", bufs=2))
        v_pool = ctx.enter_context(tc.tile_pool(name="v16", bufs=4))
        oc_pool = ctx.enter_context(tc.tile_pool(name="outcat", bufs=4))
        p_pool = ctx.enter_context(tc.tile_pool(name="pts", bufs=4))
        z_pool = ctx.enter_context(tc.tile_pool(name="zb", bufs=2))
        rz_pool = ctx.enter_context(tc.tile_pool(name="rz", bufs=2))
        bps_pool = ctx.enter_context(tc.tile_pool(name="bps", bufs=2))
        yst_pool = ctx.enter_context(tc.tile_pool(name="yst", bufs=3))
        psS = ctx.enter_context(tc.tile_pool(name="psS", bufs=2, space="PSUM"))
        psO = ctx.enter_context(tc.tile_pool(name="psO", bufs=2, space="PSUM"))
        psA = ctx.enter_context(tc.tile_pool(name="psA", bufs=2, space="PSUM"))

        # ---- weight / bias / x DMAs ----
        wq8_t = w_pool.tile([P, 8, HL * D], F8, tag="wq8")
        wk8_t = w_pool.tile([P, 8, HL * D], F8, tag="wk8")
        wv16_t = w_pool.tile([P, 8, HL * D], BF16, tag="wv16")
        for w_t, w_d in ((wq8_t, wq8_d), (wk8_t, wk8_d), (wv16_t, wv16_d)):
            nc.sync.dma_start(
                out=w_t, in_=w_d.rearrange("(n p) d -> p n d", p=P))
        wpt16_t = w_pool.tile([P, 4, C], BF16, tag="wpt16")
        nc.sync.dma_start(
            out=wpt16_t, in_=wpt16_d.rearrange("(n p) d -> p n d", p=P))
        bpb = w_pool.tile([P, C], F32, tag="bpb")
        nc.sync.dma_start(
            out=bpb,
            in_=bass.AP(tensor=bp_d.tensor, offset=0, ap=[[0, P], [1, C]]))

        x8_t = x8_pool.tile([P, 8, T], F8, tag="x8")
        x16_t = x16_pool.tile([P, 8, T], BF16, tag="x16")
        for ch in range(8):
            for th in range(2):
                tg = slice(th * 1024, (th + 1) * 1024)
                nc.sync.dma_start(
                    out=x8_t[:, ch, tg],
                    in_=x8_d[ch * P:(ch + 1) * P, tg])
        for ch in range(8):
            for th in range(2):
                tg = slice(th * 1024, (th + 1) * 1024)
                nc.sync.dma_start(
                    out=x16_t[:, ch, tg],
                    in_=x16_d[ch * P:(ch + 1) * P, tg])

        # persistent fp8 q/k per pair: [(u d), t]
        qk8 = [[qkdr_pool.tile([P, T], F8, tag="qk8", name=f"qk8_{pr}_{qk}")
                for qk in range(2)] for pr in range(4)]

        # v: 4 groups of 4 s-blocks: [s-part, sb%4, head, d | ones]
        v16_g = [v_pool.tile([P, 4, HL, D + 1], BF16, tag="v16",
                             name=f"v16_{g}") for g in range(4)]
        for g in range(4):
            nc.vector.memset(v16_g[g][:, :, :, D:], 1.0)

        outcat = [oc_pool.tile([P, T], BF16, tag="outcat", name=f"oc{q}")
                  for q in range(4)]

        ones_bc16 = w_pool.tile([1, P], BF16, tag="ones")
        nc.vector.memset(ones_bc16, 1.0)

        # ---------- emit helpers ----------
        def emit_proj_piece(pr, th, qk):
            w_t = wq8_t if qk == 0 else wk8_t
            stage = qk8[pr][qk]
            for tt in range(2):
                ps = psA.tile([P, 512], F32, tag="psA", name="qkps")
                for sub in range(2):
                    n0 = th * 1024 + tt * 512 + sub * 256
                    for cp in range(4):
                        nc.tensor.matmul(
                            ps[:, sub * 256:(sub + 1) * 256],
                            w_t[:, 2 * cp:2 * cp + 2, pr * P:(pr + 1) * P],
                            x8_t[:, 2 * cp:2 * cp + 2, n0:n0 + 256],
                            start=(cp == 0), stop=(cp == 3), perf_mode=DR)
                nc.vector.tensor_copy(
                    out=stage[:, th * 1024 + tt * 512:
                              th * 1024 + (tt + 1) * 512], in_=ps)

        def emit_v_piece(sb):
            ps = psA.tile([P, 512], F32, tag="psA", name="vps")
            for ch in range(8):
                nc.tensor.matmul(
                    ps, x16_t[:, ch, sb * P:(sb + 1) * P], wv16_t[:, ch, :],
                    start=(ch == 0), stop=(ch == 7))
            nc.vector.tensor_copy(
                out=v16_g[sb // 4][:, sb % 4, :, 0:D],
                in_=ps.rearrange("p (h d) -> p h d", h=HL))

        def emit_y_piece(m, n):
            yps = psA.tile([P, 512], F32, tag="psA", name="yps")
            for q in range(4):
                nc.tensor.matmul(
                    yps, outcat[q][:, m * P:(m + 1) * P],
                    wpt16_t[:, q, n * 512:(n + 1) * 512],
                    start=(q == 0), stop=(q == 3))
            yt = yst_pool.tile([P, 512], F32, tag="yst")
            nc.vector.tensor_add(yt, yps, bpb[:, n * 512:(n + 1) * 512])
            nc.sync.dma_start(
                out=y_d[m * P:(m + 1) * P, n * 512:(n + 1) * 512], in_=yt)

        fillers = deque()

        def pump(k=1):
            for _ in range(k):
                if fillers:
                    fillers.popleft()()

        def drain():
            while fillers:
                fillers.popleft()()

        def emit_norm(pr, j, outp):
            # Z rows (bf16) -> broadcast Z via bf16 ones-matmul -> reciprocal
            # of the broadcast on DVE -> normalize multiply on GpSimd.
            zbb = z_pool.tile([1, 2, 512], BF16, tag="zb")
            for u in range(2):
                nc.vector.tensor_copy(out=zbb[:, u, :], in_=outp[u][D:D + 1, :])
            bzp = psA.tile([P, 512], F32, tag="psA", name="bzp")
            for u in range(2):
                nc.tensor.matmul(
                    bzp[u * D:(u + 1) * D, :], ones_bc16[:, 0:D],
                    zbb[:, u, :], start=True, stop=True,
                    tile_position=(0, u * D))
            bz = bps_pool.tile([P, 512], F32, tag="bps")
            nc.vector.reciprocal_approx_fast(out=bz, in_=bzp)
            for u in range(2):
                osl = outcat[pr][u * D:(u + 1) * D, j * 512:(j + 1) * 512]
                nc.vector.tensor_copy(out=osl, in_=outp[u][0:D, :])
                nc.gpsimd.tensor_mul(osl, osl, bz[u * D:(u + 1) * D, :])

        def emit_attention_pair(pr):
            for j in range(NTT):
                nsb_j = 4 * (j + 1) if causal else NSB
                outp = [psO.tile([D + 1, 512], F32, tag="psO",
                                 name=f"outp{pr}_{j}_{u}") for u in range(2)]

                def emit_pv(i, lo, last):
                    pts = pend.pop(i)
                    for u in range(2):
                        nc.tensor.matmul(
                            outp[u][:, lo:512],
                            v16_g[i // 4][:, i % 4, pr * 2 + u, :],
                            pts[:, u, lo:512],
                            start=(i == 0), stop=last,
                            skip_group_check=True)

                pend = {}
                prev = None
                for i in range(nsb_j):
                    r = i - 4 * j if causal else -1
                    lo = max(r, 0) * P
                    scs = psS.tile([P, 2, 512], F32, tag="scs")
                    pts = p_pool.tile([P, 2, 512], BF16, tag="pts")
                    pend[i] = pts
                    for u in range(2):
                        dsl = slice(u * D, (u + 1) * D)
                        nc.tensor.matmul(
                            scs[:, u, lo:512],
                            qk8[pr][1][dsl, i * P:(i + 1) * P],
                            qk8[pr][0][dsl, j * 512 + lo:(j + 1) * 512],
                            start=True, stop=True)
                    nc.scalar.activation(
                        out=pts[:, :, lo:512], in_=scs[:, :, lo:512],
                        func=EXP, scale=SCALE_DR)
                    if causal and r >= 0:
                        nc.gpsimd.affine_select(
                            out=pts[:, :, lo:lo + P],
                            in_=pts[:, :, lo:lo + P],
                            compare_op=mybir.AluOpType.is_ge,
                            fill=0.0, base=0,
                            pattern=[[0, 2], [1, P]], channel_multiplier=-1)
                    if prev is not None:
                        emit_pv(*prev)
                        pump(1)
                    prev = (i, lo, i == nsb_j - 1)
                emit_pv(*prev)
                pump(1)
                emit_norm(pr, j, outp)
                if pr == 3:
                    jj = j
                    for m in range(4 * jj, 4 * jj + 4):
                        for n in range(2):
                            fillers.append(
                                lambda m=m, n=n: emit_y_piece(m, n))

        # ---------- schedule ----------
        # proj pair 0 + first v group up front
        for th in range(2):
            for qk in range(2):
                emit_proj_piece(0, th, qk)
        for sb in range(4):
            emit_v_piece(sb)

        for pr in range(4):
            # queue fillers: remaining v groups (pair 0 only) then next proj
            if pr == 0:
                for sb in range(4, NSB):
                    fillers.append(lambda sb=sb: emit_v_piece(sb))
            if pr < 3:
                for th in range(2):
                    for qk in range(2):
                        fillers.append(
                            lambda pr2=pr + 1, th=th, qk=qk:
                            emit_proj_piece(pr2, th, qk))
            if pr > 0:
                # everything pair pr reads must be emitted before its reads
                drain()
            emit_attention_pair(pr)
        drain()


_NC_CACHE = {}
LAST_RESULTS = None


def kernel(x, Wq, Wk, Wv, Wp, bp, is_masked, **_unused):
    global LAST_RESULTS
    x = np.asarray(x, np.float32)
    Wq = np.asarray(Wq, np.float32)
    Wk = np.asarray(Wk, np.float32)
    Wv = np.asarray(Wv, np.float32)
    Wp = np.asarray(Wp, np.float32)
    bp = np.asarray(bp, np.float32)
    causal = bool(np.asarray(is_masked).item())

    if causal not in _NC_CACHE:
        _NC_CACHE[causal] = _build(causal)
    nc = _NC_CACHE[causal]

    wq_r = Wq.transpose(1, 0, 2).reshape(C, H * D)
    wk_r = Wk.transpose(1, 0, 2).reshape(C, H * D)
    wv_r = Wv.transpose(1, 0, 2).reshape(C, H * D)
    wpt = np.ascontiguousarray(Wp.T)
    zeros = np.zeros_like(bp)

    in_maps = []
    for core in range(8):
        b, hh = core // 2, core % 2
        csl = slice(hh * HL * D, (hh + 1) * HL * D)
        xT = np.ascontiguousarray(x[b].T)
        in_maps.append({
            "x8": xT.astype(E4),
            "x16": xT.astype(BF),
            "wq8": np.ascontiguousarray(32.0 * wq_r[:, csl]).astype(E4),
            "wk8": np.ascontiguousarray(32.0 * wk_r[:, csl]).astype(E4),
            "wv16": np.ascontiguousarray(wv_r[:, csl]).astype(BF),
            "wpt16": np.ascontiguousarray(wpt[csl, :]).astype(BF),
            "bp": bp if hh == 0 else zeros,
        })

    trace = bool(int(os.environ.get("KERNEL_TRACE", "0")))
    res = run_bass_kernel_spmd(
        nc, in_maps, core_ids=list(range(8)), trace=trace)
    LAST_RESULTS = res

    y = np.empty((B, T, C), np.float32)
    for b in range(B):
        y[b] = res.results[2 * b]["y"] + res.results[2 * b + 1]["y"]
    return y
